# revision 1
# baseline (speedup 1.0000x reference)
"""Trainium2 kernel for nn_BlockLinear: gather -> per-block GEMM -> scatter-add.

Key insight: the whole op is linear in x, so gather/einsum/scatter fold into a
single dense GEMM  out[t, o] = sum_k x[t, k] * Wfull[k, o] + bias[o]  where
Wfull[k, o] = sum_{n,i,j} [input_indices[n,i]==k][output_indices[n,j]==o] * W[n,j,i].

Wfull is built on host (bincount scatter-add, exact fp64 accumulation), then the
GEMM runs on 8 NeuronCores, sharded 2D: 4 token groups x 2 out-feature groups.
Matmuls use the fp32r dtype (fp32 with 11-bit mantissa, 4x faster than fp32 on
the PE); inputs are pre-rounded on host to valid fp32r values.
"""

import numpy as np
import concourse.bacc as bacc
import concourse.mybir as mybir
import concourse.tile as tile
from concourse.bass_utils import run_bass_kernel_spmd

# problem shapes (hardcoded per contract)
B, S = 2, 2048
IN_FEATURES = 4096
OUT_FEATURES = 4096
NTOKENS = B * S                  # 4096

NCORES = 8
TG, OG = 4, 2                    # token groups x out-feature groups
T = NTOKENS // TG                # 1024 tokens per core
O = OUT_FEATURES // OG           # 2048 out features per core
P = 128
KT = IN_FEATURES // P            # 32 contraction tiles
OT = O // P                      # 16 out-feature tiles per core
NTOK = 512                       # moving free dim per matmul
TB = T // NTOK                   # 2 token blocks per core

F32R = mybir.dt.float32r
F32 = mybir.dt.float32

# knobs for test.py
TRACE = False
LAST_RESULTS = None


def round_fp32r(a: np.ndarray) -> np.ndarray:
    """Round fp32 to the nearest fp32r-representable value (11-bit mantissa)."""
    u = np.ascontiguousarray(a, dtype=np.float32).view(np.uint32)
    r = (u.astype(np.uint64) + 0x7FF + ((u >> 12) & 1)) & 0xFFFFF000
    return r.astype(np.uint32).view(np.float32)


WCHUNK = 4        # k-tiles per W DMA
WBUFS = 24        # W chunk pool bufs


def build_nc(repeats: int = 1):
    nc = bacc.Bacc()
    # xT slabs: [k][128, TB*NTOK]
    xw = nc.dram_tensor("xw", [KT, P, TB * NTOK], F32R, kind="ExternalInput")
    # W chunked [o][kc][WCHUNK, 128, 128]
    KC = KT // WCHUNK
    wrest = nc.dram_tensor(
        "wrest", [OT, KC, WCHUNK, P, P], F32R, kind="ExternalInput"
    )
    # bias in o-partition layout: [128, OT]
    bo = nc.dram_tensor("bo", [P, OT], F32, kind="ExternalInput")
    out = nc.dram_tensor("out", [OT, TB, P, NTOK], F32, kind="ExternalOutput")

    NWARM = 4  # o-groups processed k-major while the xT stream arrives

    with tile.TileContext(nc) as tc:
        with (
            tc.tile_pool(name="xw_sb", bufs=1) as xw_sb,
            tc.tile_pool(name="w_sb", bufs=WBUFS) as w_sb,
            tc.tile_pool(name="o_sb", bufs=6) as o_sb,
            tc.tile_pool(name="ps", bufs=8, space="PSUM") as ps,
        ):
            bo_t = xw_sb.tile([P, OT], F32, tag="bo")

            # PE HAM warmup: dummy matmuls on memset data fill the dead time
            # while the first DMAs land, so real matmuls start at 2.4 GHz
            dummy_sb = xw_sb.tile([P, NTOK], F32R, tag="dummy")
            nc.vector.memset(dummy_sb.bitcast(F32), 0.0)
            ps_d = ps.tile([P, NTOK], F32, tag="ps", name="ps_dummy")
            for _ in range(12):
                nc.tensor.matmul(
                    ps_d, dummy_sb[:, :P], dummy_sb, start=True, stop=True
                )

            wts = {}

            def load_w(o, rep):
                for kc in range(KC):
                    wt = w_sb.tile(
                        [P, WCHUNK, P], F32R, tag="wt", name=f"wt_{rep}_{o}_{kc}"
                    )
                    # dram [WCHUNK, 128, 128] -> sbuf [128, WCHUNK, 128];
                    # alternate issue queues to halve SP issue bursts
                    eng = nc.sync if kc % 2 == 0 else nc.scalar
                    eng.dma_start(
                        out=wt, in_=wrest[o, kc].rearrange("c k o -> k c o")
                    )
                    wts[o, kc] = wt

            # xT slabs issue k-major on the (otherwise idle) DVE queue while W
            # chunks issue on SP, interleaved in warmup consumption order
            xw_t = {}
            for kc in range(KC):
                for o in range(NWARM):
                    load_w_chunk = w_sb.tile(
                        [P, WCHUNK, P], F32R, tag="wt", name=f"wt_0_{o}_{kc}"
                    )
                    nc.sync.dma_start(
                        out=load_w_chunk,
                        in_=wrest[o, kc].rearrange("c k o -> k c o"),
                    )
                    wts[o, kc] = load_w_chunk
                    # interleave xw issues between W issues so neither stream
                    # blocks the other's first arrivals
                    k = kc * WCHUNK + o
                    if o < WCHUNK:
                        t = xw_sb.tile([P, TB * NTOK], F32R, tag=f"xw_{k}")
                        nc.scalar.dma_start(out=t, in_=xw[k])
                        xw_t[k] = t
                for k in range(kc * WCHUNK, (kc + 1) * WCHUNK):
                    if k not in xw_t:
                        t = xw_sb.tile([P, TB * NTOK], F32R, tag=f"xw_{k}")
                        nc.scalar.dma_start(out=t, in_=xw[k])
                        xw_t[k] = t
                if kc == 0:
                    # bias load is only needed by the drains, ~60us later;
                    # keep its issue slot off the critical path
                    nc.sync.dma_start(out=bo_t, in_=bo[:, :])

            def drain(o, tb, psum):
                o_t = o_sb.tile([P, NTOK], F32, tag="ot", name=f"ot_{o}_{tb}")
                # psum -> sbuf with per-partition bias add; alternate engines
                # so consecutive drains run in parallel
                if (o * TB + tb) % 2 == 0:
                    nc.scalar.add(o_t, psum, bo_t[:, o : o + 1])
                else:
                    nc.vector.tensor_scalar_add(o_t, psum, bo_t[:, o : o + 1])
                nc.scalar.dma_start(out=out[o, tb, :, :], in_=o_t)

            def mm_group(o, rep):
                psums = {
                    tb: ps.tile([P, NTOK], F32, tag="ps", name=f"ps_{rep}_{o}_{tb}")
                    for tb in range(TB)
                }
                for k in range(KT):
                    lhsT = wts[o, k // WCHUNK][:, k % WCHUNK]
                    for tb in range(TB):
                        nc.tensor.matmul(
                            psums[tb],
                            lhsT,
                            xw_t[k][:, tb * NTOK : (tb + 1) * NTOK],
                            start=(k == 0),
                            stop=(k == KT - 1),
                        )
                for tb in range(TB):
                    drain(o, tb, psums[tb])

            for _rep in range(repeats):
                if _rep == 0:
                    # warmup phase: k-major over NWARM o-groups x TB token
                    # blocks (all 8 psum banks) -> 8 matmuls per arriving
                    # xT k-slab, keeping the PE busy while xT streams in
                    psums = {
                        (o, tb): ps.tile(
                            [P, NTOK], F32, tag="ps", name=f"psw_{o}_{tb}"
                        )
                        for o in range(NWARM)
                        for tb in range(TB)
                    }
                    for k in range(KT - WCHUNK):
                        for o in range(NWARM):
                            lhsT = wts[o, k // WCHUNK][:, k % WCHUNK]
                            for tb in range(TB):
                                nc.tensor.matmul(
                                    psums[o, tb],
                                    lhsT,
                                    xw_t[k][:, tb * NTOK : (tb + 1) * NTOK],
                                    start=(k == 0),
                                    stop=False,
                                )
                    # last k-window o-major with immediate drains, so psum
                    # banks free one o-group at a time and the steady phase
                    # starts while the rest of the warmup finishes
                    for o in range(NWARM):
                        for k in range(KT - WCHUNK, KT):
                            lhsT = wts[o, k // WCHUNK][:, k % WCHUNK]
                            for tb in range(TB):
                                nc.tensor.matmul(
                                    psums[o, tb],
                                    lhsT,
                                    xw_t[k][:, tb * NTOK : (tb + 1) * NTOK],
                                    start=False,
                                    stop=(k == KT - 1),
                                )
                        for tb in range(TB):
                            drain(o, tb, psums[o, tb])
                    o_start = NWARM
                else:
                    o_start = 0
                for o in range(o_start, OT):
                    load_w(o, _rep)
                    mm_group(o, _rep)
    nc.finalize()
    return nc


_NC = None


def _get_nc():
    global _NC
    if _NC is None:
        _NC = build_nc()
    return _NC


def _build_wfull(weights, input_indices, output_indices):
    """Wfull[k, o] = sum over blocks/dups of weights[n, j, i]."""
    ii = np.asarray(input_indices).astype(np.int64)     # [NBLK, BI]
    oi = np.asarray(output_indices).astype(np.int64)    # [NBLK, BO]
    w = np.asarray(weights, dtype=np.float64)           # [NBLK, BO, BI]
    flat = (ii[:, :, None] * OUT_FEATURES + oi[:, None, :]).ravel()  # [n, i, j]
    vals = np.ascontiguousarray(np.swapaxes(w, 1, 2)).ravel()        # [n, i, j]
    wfull = np.bincount(flat, weights=vals, minlength=IN_FEATURES * OUT_FEATURES)
    return wfull.reshape(IN_FEATURES, OUT_FEATURES).astype(np.float32)


def prepare_in_maps(x, weights, bias, input_indices, output_indices):
    x = np.asarray(x, dtype=np.float32)
    bias = np.asarray(bias, dtype=np.float32)

    wfull = round_fp32r(_build_wfull(weights, input_indices, output_indices))
    xr = round_fp32r(x.reshape(NTOKENS, IN_FEATURES))

    in_maps = []
    for c in range(NCORES):
        tg, og = divmod(c, OG)
        xT = np.ascontiguousarray(xr[tg * T : (tg + 1) * T, :].T)   # [K, T]
        xw = np.ascontiguousarray(xT.reshape(KT, P, T))
        wr = np.ascontiguousarray(
            wfull[:, og * O : (og + 1) * O]
            .reshape(KT, P, OT, P)
            .transpose(2, 0, 1, 3)
        ).reshape(OT, KT // WCHUNK, WCHUNK, P, P)
        # bias in o-partition layout [128, OT]; full fp32 (added exactly on ACT)
        bo = np.ascontiguousarray(
            bias[og * O : (og + 1) * O].reshape(OT, P).T
        )
        in_maps.append({"xw": xw, "wrest": wr, "bo": bo})
    return in_maps


def assemble_output(core_outs):
    full = np.empty((NTOKENS, OUT_FEATURES), np.float32)
    for c in range(NCORES):
        tg, og = divmod(c, OG)
        o4 = np.asarray(core_outs[c])                    # [OT, TB, P, NTOK]
        blk = o4.transpose(1, 3, 0, 2).reshape(T, O)     # [t, o]
        full[tg * T : (tg + 1) * T, og * O : (og + 1) * O] = blk
    return full.reshape(B, S, OUT_FEATURES)


def kernel(x, weights, bias, input_indices, output_indices):
    global LAST_RESULTS
    in_maps = prepare_in_maps(x, weights, bias, input_indices, output_indices)
    nc = _get_nc()
    res = run_bass_kernel_spmd(nc, in_maps, list(range(NCORES)))
    LAST_RESULTS = res
    return assemble_output([res.results[c]["out"] for c in range(NCORES)])



# revision 3
# speedup vs baseline: 1.3210x; 1.3210x over previous
"""Trainium2 kernel for nn_BlockLinear: gather -> per-block GEMM -> scatter-add.

The whole op is linear in x, so gather/einsum/scatter fold into one dense GEMM
out[t, o] = sum_k x[t, k] * Wfull[k, o] + bias[o], with Wfull built on host
(bincount scatter-add, exact fp64 accumulation). The GEMM runs on 8
NeuronCores, sharded 2D: 4 token groups x 2 out-feature groups.

Mixed-precision contraction split (rel-err budget 2e-2, measured 1.8e-2):
the first N8*256 of K runs as fp8(e4m3) DoubleRow matmuls (256-contraction per
instruction, 2x PE throughput), the remaining K in bf16. Operands are
pre-scaled by powers of two (x*2^5, w*2^10) so both parts accumulate in one
fp32 PSUM group; drains rescale by 2^-15 and add the bias in one fused op.
"""

import numpy as np
import ml_dtypes
import concourse.bacc as bacc
import concourse.mybir as mybir
import concourse.tile as tile
from concourse.bass_utils import run_bass_kernel_spmd

# problem shapes (hardcoded per contract)
B, S = 2, 2048
IN_FEATURES = 4096
OUT_FEATURES = 4096
NTOKENS = B * S                  # 4096

NCORES = 8
TG, OG = 4, 2                    # token groups x out-feature groups
T = NTOKENS // TG                # 1024 tokens per core
O = OUT_FEATURES // OG           # 2048 out features per core
P = 128
KT = IN_FEATURES // P            # 32 contraction tiles
OT = O // P                      # 16 out-feature tiles per core
NTOK = 512                       # moving free dim per matmul
TB = T // NTOK                   # 2 token blocks per core

N8 = 6                           # fp8 DoubleRow pairs (256 K each)
K8 = N8 * 2 * P                  # 1536 K contracted in fp8
KB = KT - N8 * 2                 # 20 bf16 k-tiles
WCHUNK = 4                       # bf16 k-tiles per W DMA chunk
KC = KB // WCHUNK                # 5 chunks

SX = 32.0                        # x pre-scale (power of 2)
SW = 1024.0                      # w pre-scale (power of 2)
SCALE_OUT = 1.0 / (SX * SW)      # exact 2^-15

F32 = mybir.dt.float32
BF16 = mybir.dt.bfloat16
FP8 = mybir.dt.float8e4
DR = mybir.MatmulPerfMode.DoubleRow
NP_FP8 = ml_dtypes.float8_e4m3   # TRN FP8_EXP4: max normal 240
NP_BF16 = ml_dtypes.bfloat16

# knobs for test.py
TRACE = False
LAST_RESULTS = None

# contraction units per (o, tb) psum chain: N8 DoubleRow + KB bf16
UNITS = [("dr", i) for i in range(N8)] + [("bf", i) for i in range(KB)]
NU = len(UNITS)                  # 26
LASTW = 4                        # trailing units drained o-major in warmup
NWARM = 4                        # o-groups processed k-major during warmup


def build_nc():
    nc = bacc.Bacc()
    x8 = nc.dram_tensor("x8", [N8, P, 2, T], FP8, kind="ExternalInput")
    xb = nc.dram_tensor("xb", [KB, P, T], BF16, kind="ExternalInput")
    w8 = nc.dram_tensor("w8", [OT, P, N8, 2, P], FP8, kind="ExternalInput")
    wb = nc.dram_tensor("wb", [OT, KC, P, WCHUNK, P], BF16, kind="ExternalInput")
    bo = nc.dram_tensor("bo", [P, OT], F32, kind="ExternalInput")
    out = nc.dram_tensor("out", [OT, TB, P, NTOK], F32, kind="ExternalOutput")

    with tile.TileContext(nc) as tc:
        with (
            tc.tile_pool(name="x_sb", bufs=1) as x_sb,
            tc.tile_pool(name="w8_sb", bufs=6) as w8_sb,
            tc.tile_pool(name="wb_sb", bufs=15) as wb_sb,
            tc.tile_pool(name="o_sb", bufs=6) as o_sb,
            tc.tile_pool(name="ps", bufs=8, space="PSUM") as ps,
        ):
            bo_t = x_sb.tile([P, OT], F32, tag="bo")

            # PE HAM warmup: dummy matmuls on memset data fill the dead time
            # while the first DMAs land, so real matmuls start at 2.4 GHz
            dummy_sb = x_sb.tile([P, NTOK], BF16, tag="dummy")
            nc.vector.memset(dummy_sb.bitcast(F32), 0.0)
            ps_d = ps.tile([P, NTOK], F32, tag="ps", name="ps_dummy")
            for _ in range(12):
                nc.tensor.matmul(
                    ps_d, dummy_sb[:, :P], dummy_sb, start=True, stop=True
                )

            w8t, wbt = {}, {}

            def load_w8(o):
                t = w8_sb.tile([P, N8, 2, P], FP8, tag="w8t", name=f"w8_{o}")
                nc.sync.dma_start(out=t, in_=w8[o])
                w8t[o] = t

            def load_wb(o):
                for kc in range(KC):
                    t = wb_sb.tile(
                        [P, WCHUNK, P], BF16, tag="wbt", name=f"wb_{o}_{kc}"
                    )
                    eng = nc.sync if kc % 2 == 0 else nc.scalar
                    eng.dma_start(out=t, in_=wb[o, kc])
                    wbt[o, kc] = t

            # x slabs stream on the (otherwise idle) gpsimd queue
            x8_t, xb_t = {}, {}

            def load_x8(i):
                t = x_sb.tile([P, 2, T], FP8, tag=f"x8_{i}")
                nc.gpsimd.dma_start(out=t, in_=x8[i])
                x8_t[i] = t

            def load_xb(i):
                t = x_sb.tile([P, T], BF16, tag=f"xb_{i}")
                nc.gpsimd.dma_start(out=t, in_=xb[i])
                xb_t[i] = t

            def unit_mm(psum, o, u, tb, start, stop):
                kind, i = UNITS[u]
                if kind == "dr":
                    nc.tensor.matmul(
                        psum,
                        w8t[o][:, i],
                        x8_t[i][:, :, tb * NTOK : (tb + 1) * NTOK],
                        start=start,
                        stop=stop,
                        perf_mode=DR,
                    )
                else:
                    nc.tensor.matmul(
                        psum,
                        wbt[o, i // WCHUNK][:, i % WCHUNK],
                        xb_t[i][:, tb * NTOK : (tb + 1) * NTOK],
                        start=start,
                        stop=stop,
                    )

            def drain(o, tb, psum):
                o_t = o_sb.tile([P, NTOK], F32, tag="ot", name=f"ot_{o}_{tb}")
                # psum * 2^-15 + bias in one fused op; alternate engines so
                # consecutive drains run in parallel
                if (o * TB + tb) % 2 == 0:
                    nc.scalar.activation(
                        o_t,
                        psum,
                        mybir.ActivationFunctionType.Identity,
                        bias=bo_t[:, o : o + 1],
                        scale=SCALE_OUT,
                    )
                else:
                    nc.vector.tensor_scalar(
                        o_t,
                        psum,
                        SCALE_OUT,
                        bo_t[:, o : o + 1],
                        mybir.AluOpType.mult,
                        mybir.AluOpType.add,
                    )
                nc.scalar.dma_start(out=out[o, tb, :, :], in_=o_t)

            # ---- DMA issue for the warmup span ----
            # warmup W first on SP so the first matmuls aren't blocked
            for o in range(NWARM):
                load_w8(o)
            # x slabs in consumption order on DVE; bf16 W chunks interleave
            for i in range(N8):
                load_x8(i)
            load_wb(0)
            nc.sync.dma_start(out=bo_t, in_=bo[:, :])
            for i in range(KB):
                load_xb(i)
                if i < (NWARM - 1) * KC:
                    o, kc = divmod(i, KC)
                    o += 1
                    t = wb_sb.tile(
                        [P, WCHUNK, P], BF16, tag="wbt", name=f"wb_{o}_{kc}"
                    )
                    eng = nc.sync if kc % 2 == 0 else nc.scalar
                    eng.dma_start(out=t, in_=wb[o, kc])
                    wbt[o, kc] = t

            # ---- warmup: k-major over NWARM o-groups x TB token blocks ----
            psums = {
                (o, tb): ps.tile([P, NTOK], F32, tag="ps", name=f"psw_{o}_{tb}")
                for o in range(NWARM)
                for tb in range(TB)
            }
            for u in range(NU - LASTW):
                for o in range(NWARM):
                    for tb in range(TB):
                        unit_mm(psums[o, tb], o, u, tb, u == 0, False)
            # last units o-major with immediate drains, so psum banks free one
            # o-group at a time and the steady phase starts while the rest of
            # the warmup finishes
            for o in range(NWARM):
                for u in range(NU - LASTW, NU):
                    for tb in range(TB):
                        unit_mm(psums[o, tb], o, u, tb, False, u == NU - 1)
                for tb in range(TB):
                    drain(o, tb, psums[o, tb])

            # ---- steady: o-major ----
            for o in range(NWARM, OT):
                load_w8(o)
                load_wb(o)
                psg = {
                    tb: ps.tile([P, NTOK], F32, tag="ps", name=f"ps_{o}_{tb}")
                    for tb in range(TB)
                }
                for u in range(NU):
                    for tb in range(TB):
                        unit_mm(psg[tb], o, u, tb, u == 0, u == NU - 1)
                for tb in range(TB):
                    drain(o, tb, psg[tb])
    nc.finalize()
    return nc


_NC = None


def _get_nc():
    global _NC
    if _NC is None:
        _NC = build_nc()
    return _NC


def _build_wfull(weights, input_indices, output_indices):
    """Wfull[k, o] = sum over blocks/dups of weights[n, j, i]."""
    ii = np.asarray(input_indices).astype(np.int64)     # [NBLK, BI]
    oi = np.asarray(output_indices).astype(np.int64)    # [NBLK, BO]
    w = np.asarray(weights, dtype=np.float64)           # [NBLK, BO, BI]
    flat = (ii[:, :, None] * OUT_FEATURES + oi[:, None, :]).ravel()  # [n, i, j]
    vals = np.ascontiguousarray(np.swapaxes(w, 1, 2)).ravel()        # [n, i, j]
    wfull = np.bincount(flat, weights=vals, minlength=IN_FEATURES * OUT_FEATURES)
    return wfull.reshape(IN_FEATURES, OUT_FEATURES)


def _to_fp8(a):
    return np.clip(a, -240.0, 240.0).astype(NP_FP8)


def prepare_in_maps(x, weights, bias, input_indices, output_indices):
    x = np.asarray(x, dtype=np.float32).reshape(NTOKENS, IN_FEATURES)
    bias = np.asarray(bias, dtype=np.float32)
    wfull = _build_wfull(weights, input_indices, output_indices)

    # quantize once globally (scales are powers of two; folded out in drain)
    x8_full = _to_fp8(x[:, :K8].astype(np.float64) * SX)         # [NT, K8]
    xb_full = (x[:, K8:] * np.float32(SX)).astype(NP_BF16)       # [NT, K-K8]
    w8_full = _to_fp8(wfull[:K8, :] * SW)                        # [K8, OF]
    wb_full = (wfull[K8:, :] * SW).astype(NP_BF16)               # [K-K8, OF]

    in_maps = []
    for c in range(NCORES):
        tg, og = divmod(c, OG)
        tsl = slice(tg * T, (tg + 1) * T)
        osl = slice(og * O, (og + 1) * O)
        # x8: [N8, P, 2, T] ; k = (2*kk + j)*128 + p
        x8c = np.ascontiguousarray(
            x8_full[tsl].T.reshape(N8, 2, P, T).transpose(0, 2, 1, 3)
        )
        # xb: [KB, P, T]
        xbc = np.ascontiguousarray(xb_full[tsl].T.reshape(KB, P, T))
        # w8: [OT, P, N8, 2, P]
        w8c = np.ascontiguousarray(
            w8_full[:, osl].reshape(N8, 2, P, OT, P).transpose(3, 2, 0, 1, 4)
        )
        # wb: [OT, KC, P, WCHUNK, P]
        wbc = np.ascontiguousarray(
            wb_full[:, osl].reshape(KC, WCHUNK, P, OT, P).transpose(3, 0, 2, 1, 4)
        )
        boc = np.ascontiguousarray(bias[osl].reshape(OT, P).T)
        in_maps.append({"x8": x8c, "xb": xbc, "w8": w8c, "wb": wbc, "bo": boc})
    return in_maps


def assemble_output(core_outs):
    full = np.empty((NTOKENS, OUT_FEATURES), np.float32)
    for c in range(NCORES):
        tg, og = divmod(c, OG)
        o4 = np.asarray(core_outs[c])                    # [OT, TB, P, NTOK]
        blk = o4.transpose(1, 3, 0, 2).reshape(T, O)     # [t, o]
        full[tg * T : (tg + 1) * T, og * O : (og + 1) * O] = blk
    return full.reshape(B, S, OUT_FEATURES)


def kernel(x, weights, bias, input_indices, output_indices):
    global LAST_RESULTS
    in_maps = prepare_in_maps(x, weights, bias, input_indices, output_indices)
    nc = _get_nc()
    res = run_bass_kernel_spmd(nc, in_maps, list(range(NCORES)))
    LAST_RESULTS = res
    return assemble_output([res.results[c]["out"] for c in range(NCORES)])
